# revision 9
# baseline (speedup 1.0000x reference)
"""CARFAC cell kernel for 8 TRN2 NeuronCores.

Math: y[b,c,n] is the linear recurrence a[n+1] = f[n+1]*a[n] + g[n+1]
(computed exactly with the DVE's tensor_tensor_scan instruction — the
reference's cumsum-of-logs + triangular-matmul expansion is just a
parallel-friendly expression of the same recurrence), followed by
`steps` rounds of a symmetric-padded 3-tap FIR across channels.

Key identity used for the smoothing stage: symmetric (half-sample
reflect) padding commutes with a symmetric FIR, so applying the 3-tap
kernel `steps` times equals ONE conv with the `steps`-fold
self-convolution of the kernel (17 taps for steps=8) applied on the
reflect-extended signal. That collapses to a single [C x C] matrix W
(banded + boundary-folded), i.e. one TensorEngine matmul.

Sharding: 8 cores = 2 batches x 4 channel-quarters. Each core loads its
owned ~18 channels plus an 8-channel halo (<=34 rows of f/g), scans the
recurrence for all loaded rows, and applies its [34 x 18] slice of W
(halo selection + reflection encoded host-side in the weights). No
cross-core communication of any kind.
"""

import numpy as np

B, C, N = 2, 71, 1024
NCORES = 8
QPB = 4  # channel-quarters per batch element
HALO = 8  # channel reach of the smoothing: steps * (ksz-1)//2
ROWS = 34  # rows loaded per core: own(<=18) + up to 2*HALO, padded
OWN = 18  # owned output channels per core (last quarter uses 17)

_OWN_LO = [0, 18, 36, 54]
_OWN_SZ = [18, 18, 18, 17]

_PROGRAM = None


PACK = 2 * N + 1 + OWN  # f | g | a0 | w packed along the free axis


def _build_program():
    """Raw Bass (no Tile): 11 instructions, manual semaphores.

    Tile's tail drain attaches one sync-wait per live proc and the HW caps
    waits per instruction; raw Bass keeps every instruction at <=1 wait.
    """
    import concourse.bass as bass
    import concourse.mybir as mybir

    f32 = mybir.dt.float32
    mult, add = mybir.AluOpType.mult, mybir.AluOpType.add
    nc = bass.Bass()
    in_loc = nc.declare_dram_parameter("in_loc", [ROWS, PACK], f32, isOutput=False)
    out_loc = nc.declare_dram_parameter("out_loc", [OWN, N], f32, isOutput=True)

    HALF = 512  # one PSUM bank of fp32 per matmul

    with (
        nc.sbuf_tensor([ROWS, PACK], f32) as it,
        nc.sbuf_tensor([ROWS, N], f32) as yt,
        nc.sbuf_tensor([OWN, N], f32) as ot,
        nc.psum_tensor([OWN, HALF], f32) as ps0,
        nc.psum_tensor([OWN, HALF], f32) as ps1,
        nc.semaphore("dma_sem") as dma_sem,
        nc.semaphore("v_sem") as v_sem,
        nc.semaphore("p_sem") as p_sem,
        nc.Block() as block,
    ):
        ft = it[:, 0:N]
        gt = it[:, N : 2 * N]
        a0t = it[:, 2 * N : 2 * N + 1]
        wt = it[:, 2 * N + 1 : PACK]
        ps = [ps0, ps1]

        # All DMAs on the gpsimd SWDGE path: one SWDGE transfer fans out
        # across all 16 SDMA engines (~200+ GB/s), while a HWDGE (sync)
        # DMA_DIRECT2D runs its descriptors on ONE engine (~26 GB/s —
        # measured 10.5 us for the 281 KB input load).
        @block.gpsimd
        def _(g):
            g.dma_start(out=it[:, :], in_=in_loc[:, :]).then_inc(dma_sem, 16)
            g.wait_ge(v_sem, 3)  # PSUM->SBUF copy of half 0 done
            g.dma_start(out=out_loc[:, :HALF], in_=ot[:, :HALF]).then_inc(dma_sem, 16)
            g.wait_ge(v_sem, 4)  # copy of half 1 done
            g.dma_start(out=out_loc[:, HALF:], in_=ot[:, HALF:]).then_inc(dma_sem, 16)
            g.wait_ge(dma_sem, 48)  # output landed before kernel end

        @block.vector
        def _(vector):
            vector.wait_ge(dma_sem, 16)
            # Scan in halves so the first matmul overlaps the second half;
            # the chain passes the carry via initial=prev_out[:, -1:].
            vector.tensor_tensor_scan(
                yt[:, :HALF], ft[:, :HALF], gt[:, :HALF], a0t, op0=mult, op1=add
            ).then_inc(v_sem, 1)
            vector.wait_ge(v_sem, 1)  # carry element readable (race detector)
            vector.tensor_tensor_scan(
                yt[:, HALF:],
                ft[:, HALF:],
                gt[:, HALF:],
                yt[:, HALF - 1 : HALF],
                op0=mult,
                op1=add,
            ).then_inc(v_sem, 1)
            for h in range(2):
                vector.wait_ge(p_sem, h + 1)
                vector.tensor_copy(
                    ot[:, h * HALF : (h + 1) * HALF], ps[h][:, :]
                ).then_inc(v_sem, 1)

        @block.tensor
        def _(tensor):
            for h in range(2):
                # v_sem >= h+1: scan half h done; implies the input DMA
                # (incl. the weight slice) completed, since the DVE gated
                # its scans on dma_sem.
                tensor.wait_ge(v_sem, h + 1)
                tensor.matmul(
                    ps[h][:, :],
                    wt,
                    yt[:, h * HALF : (h + 1) * HALF],
                    start=True,
                    stop=True,
                ).then_inc(p_sem, 1)

    return nc


def _conv_matrix(kernel: np.ndarray, steps: int) -> np.ndarray:
    """[C, C] matrix equivalent to `steps` rounds of symmetric-pad conv."""
    eff = np.array([1.0], np.float64)
    for _ in range(steps):
        eff = np.convolve(eff, kernel.astype(np.float64))
    h = (len(eff) - 1) // 2
    assert h <= HALO, f"kernel reach {h} exceeds layout halo {HALO}"
    W = np.zeros((C, C), np.float64)
    for c in range(C):
        for d in range(-h, h + 1):
            idx = c + d
            if idx < 0:
                idx = -1 - idx
            if idx >= C:
                idx = 2 * C - 1 - idx
            W[idx, c] += eff[d + h]
    return W.astype(np.float32)


LAST_RESULT = None  # BassKernelResults of the most recent run (for test.py)
TRACE = False  # set True (e.g. by test.py) to capture an NTFF profile


def kernel(a_0, f, g, kernel, steps):
    global _PROGRAM, LAST_RESULT
    from concourse.bass_utils import run_bass_kernel_spmd

    a_0 = np.asarray(a_0, np.float32)
    f = np.asarray(f, np.float32)
    g = np.asarray(g, np.float32)
    W = _conv_matrix(np.asarray(kernel), int(steps))

    in_maps = []
    meta = []
    for core in range(NCORES):
        b, q = divmod(core, QPB)
        lo, sz = _OWN_LO[q], _OWN_SZ[q]
        r0 = max(0, lo - HALO)
        r1 = min(C, lo + sz + HALO)
        nr = r1 - r0

        in_loc = np.zeros((ROWS, PACK), np.float32)
        in_loc[:, :N] = 0.5  # benign f for padded rows
        in_loc[:nr, 0:N] = f[b, r0:r1]
        in_loc[:nr, N : 2 * N] = g[b, r0:r1]
        in_loc[:nr, 2 * N] = a_0[b, r0:r1]
        in_loc[:nr, 2 * N + 1 : 2 * N + 1 + sz] = W[r0:r1, lo : lo + sz]

        in_maps.append({"in_loc": in_loc})
        meta.append((b, lo, sz))

    if _PROGRAM is None:
        _PROGRAM = _build_program()

    res = run_bass_kernel_spmd(
        _PROGRAM, in_maps, core_ids=list(range(NCORES)), trace=TRACE
    )
    LAST_RESULT = res

    out = np.empty((B, C, N), np.float32)
    for core, (b, lo, sz) in enumerate(meta):
        out[b, lo : lo + sz] = res.results[core]["out_loc"][:sz]
    return out


# revision 10
# speedup vs baseline: 1.2368x; 1.2368x over previous
"""CARFAC cell kernel for 8 TRN2 NeuronCores.

Math: y[b,c,n] is the linear recurrence a[n+1] = f[n+1]*a[n] + g[n+1]
(computed exactly with the DVE's tensor_tensor_scan instruction — the
reference's cumsum-of-logs + triangular-matmul expansion is just a
parallel-friendly expression of the same recurrence), followed by
`steps` rounds of a symmetric-padded 3-tap FIR across channels.

Key identity for the smoothing stage: half-sample symmetric padding
commutes with a symmetric FIR, so applying the 3-tap kernel `steps`
times equals ONE conv with the `steps`-fold self-convolution of the
kernel (17 taps for steps=8) on the reflect-extended signal. That
collapses to a single [C x C] matrix W (banded + boundary-folded),
i.e. one TensorEngine matmul.

Sharding: 8 cores = 2 batches x 4 channel-quarters. Each core loads its
owned ~18 channels plus an 8-channel halo (<=34 rows of f/g), scans the
recurrence for all loaded rows, and applies its [34 x 18] slice of W
(halo selection + reflection encoded host-side in the weights). No
cross-core communication of any kind.

Performance notes (from neuron-profile traces):
- A dynamic DMA's descriptors are processed by ONE SDMA engine
  (~27 GB/s = one SBUF port); the sync, scalar and gpsimd DGE paths are
  independent, so large transfers are split across all three engines.
- Raw Bass (no Tile, no Block): Tile's tail drain exceeds the HW's
  per-instruction sync-wait cap, and Block's exit all-engine barrier
  costs ~4 us of pure epilogue.
- enable_partition_id=False drops a ~1 us per-engine register load
  from the preamble.
"""

import numpy as np

B, C, N = 2, 71, 1024
NCORES = 8
QPB = 4  # channel-quarters per batch element
HALO = 8  # channel reach of the smoothing: steps * (ksz-1)//2
ROWS = 34  # rows loaded per core: own(<=18) + up to 2*HALO, padded
OWN = 18  # owned output channels per core (last quarter uses 17)

_OWN_LO = [0, 18, 36, 54]
_OWN_SZ = [18, 18, 18, 17]

PACK = 2 * N + 1 + OWN  # f | g | a0 | w packed along the free axis

_PROGRAM = None


def _build_program():
    import concourse.bass as bass
    import concourse.mybir as mybir

    f32 = mybir.dt.float32
    mult, add = mybir.AluOpType.mult, mybir.AluOpType.add
    nc = bass.Bass(enable_partition_id=False)
    in_loc = nc.declare_dram_parameter("in_loc", [ROWS, PACK], f32, isOutput=False)
    out_loc = nc.declare_dram_parameter("out_loc", [OWN, N], f32, isOutput=True)

    HALF = 512  # one PSUM bank of fp32 per matmul

    with (
        nc.sbuf_tensor([ROWS, PACK], f32) as it,
        nc.sbuf_tensor([ROWS, N], f32) as yt,
        nc.sbuf_tensor([OWN, N], f32) as ot,
        nc.psum_tensor([OWN, HALF], f32) as ps0,
        nc.psum_tensor([OWN, HALF], f32) as ps1,
        nc.semaphore("dma_sem") as dma_sem,
        nc.semaphore("v_sem") as v_sem,
        nc.semaphore("p_sem") as p_sem,
    ):
        ft = it[:, 0:N]
        gt = it[:, N : 2 * N]
        a0t = it[:, 2 * N : 2 * N + 1]
        wt = it[:, 2 * N + 1 : PACK]
        ps = [ps0, ps1]

        # Input load, split across the three independent DGE paths.
        IN_SPLITS = [("sync", 0, 12), ("scalar", 12, 24), ("gpsimd", 24, 34)]
        for eng, r0, r1 in IN_SPLITS:
            getattr(nc, eng).dma_start(
                out=it[r0:r1, :], in_=in_loc[r0:r1, :]
            ).then_inc(dma_sem, 16)
        IN_DONE = 16 * len(IN_SPLITS)

        # Recurrence scan in halves, chained via initial=prev_out[:, -1:],
        # so the first matmul overlaps the second half's scan.
        nc.vector.wait_ge(dma_sem, IN_DONE)
        nc.vector.tensor_tensor_scan(
            yt[:, :HALF], ft[:, :HALF], gt[:, :HALF], a0t, op0=mult, op1=add
        ).then_inc(v_sem, 1)
        nc.vector.wait_ge(v_sem, 1)  # carry element readable (race detector)
        nc.vector.tensor_tensor_scan(
            yt[:, HALF:],
            ft[:, HALF:],
            gt[:, HALF:],
            yt[:, HALF - 1 : HALF],
            op0=mult,
            op1=add,
        ).then_inc(v_sem, 1)

        # Smoothing matmul per half; v_sem >= h+1 implies the input DMA
        # completed (the DVE gated its scans on dma_sem), so wt is ready.
        for h in range(2):
            nc.tensor.wait_ge(v_sem, h + 1)
            nc.tensor.matmul(
                ps[h][:, :],
                wt,
                yt[:, h * HALF : (h + 1) * HALF],
                start=True,
                stop=True,
            ).then_inc(p_sem, 1)
            nc.vector.wait_ge(p_sem, h + 1)
            nc.vector.tensor_copy(
                ot[:, h * HALF : (h + 1) * HALF], ps[h][:, :]
            ).then_inc(v_sem, 1)

        # Store, split across the three DGE paths (all gated on both copies).
        OUT_SPLITS = [("sync", 0, 6), ("scalar", 6, 12), ("gpsimd", 12, 18)]
        for eng, r0, r1 in OUT_SPLITS:
            e = getattr(nc, eng)
            e.wait_ge(v_sem, 4)
            e.dma_start(out=out_loc[r0:r1, :], in_=ot[r0:r1, :]).then_inc(dma_sem, 16)
        nc.sync.wait_ge(dma_sem, IN_DONE + 16 * len(OUT_SPLITS))

    return nc


def _conv_matrix(kernel: np.ndarray, steps: int) -> np.ndarray:
    """[C, C] matrix equivalent to `steps` rounds of symmetric-pad conv."""
    eff = np.array([1.0], np.float64)
    for _ in range(steps):
        eff = np.convolve(eff, kernel.astype(np.float64))
    h = (len(eff) - 1) // 2
    assert h <= HALO, f"kernel reach {h} exceeds layout halo {HALO}"
    W = np.zeros((C, C), np.float64)
    for c in range(C):
        for d in range(-h, h + 1):
            idx = c + d
            if idx < 0:
                idx = -1 - idx
            if idx >= C:
                idx = 2 * C - 1 - idx
            W[idx, c] += eff[d + h]
    return W.astype(np.float32)


LAST_RESULT = None  # BassKernelResults of the most recent run (for test.py)
TRACE = False  # set True (e.g. by test.py) to capture an NTFF profile


def kernel(a_0, f, g, kernel, steps):
    global _PROGRAM, LAST_RESULT
    from concourse.bass_utils import run_bass_kernel_spmd

    a_0 = np.asarray(a_0, np.float32)
    f = np.asarray(f, np.float32)
    g = np.asarray(g, np.float32)
    W = _conv_matrix(np.asarray(kernel), int(steps))

    in_maps = []
    meta = []
    for core in range(NCORES):
        b, q = divmod(core, QPB)
        lo, sz = _OWN_LO[q], _OWN_SZ[q]
        r0 = max(0, lo - HALO)
        r1 = min(C, lo + sz + HALO)
        nr = r1 - r0

        in_loc = np.zeros((ROWS, PACK), np.float32)
        in_loc[:, :N] = 0.5  # benign f for padded rows
        in_loc[:nr, 0:N] = f[b, r0:r1]
        in_loc[:nr, N : 2 * N] = g[b, r0:r1]
        in_loc[:nr, 2 * N] = a_0[b, r0:r1]
        in_loc[:nr, 2 * N + 1 : 2 * N + 1 + sz] = W[r0:r1, lo : lo + sz]

        in_maps.append({"in_loc": in_loc})
        meta.append((b, lo, sz))

    if _PROGRAM is None:
        _PROGRAM = _build_program()

    res = run_bass_kernel_spmd(
        _PROGRAM, in_maps, core_ids=list(range(NCORES)), trace=TRACE
    )
    LAST_RESULT = res

    out = np.empty((B, C, N), np.float32)
    for core, (b, lo, sz) in enumerate(meta):
        out[b, lo : lo + sz] = res.results[core]["out_loc"][:sz]
    return out


# revision 13
# speedup vs baseline: 1.3881x; 1.1223x over previous
"""CARFAC cell kernel for 8 TRN2 NeuronCores.

Math: y[b,c,n] is the linear recurrence a[n+1] = f[n+1]*a[n] + g[n+1]
(computed exactly with the DVE's tensor_tensor_scan instruction — the
reference's cumsum-of-logs + triangular-matmul expansion is just a
parallel-friendly expression of the same recurrence), followed by
`steps` rounds of a symmetric-padded 3-tap FIR across channels.

Key identity for the smoothing stage: half-sample symmetric padding
commutes with a symmetric FIR, so applying the 3-tap kernel `steps`
times equals ONE conv with the `steps`-fold self-convolution of the
kernel (17 taps for steps=8) on the reflect-extended signal. That
collapses to a single [C x C] matrix W (banded + boundary-folded),
i.e. one TensorEngine matmul.

Sharding: 8 cores = 2 batches x 4 channel-quarters. Each core loads its
owned ~18 channels plus an 8-channel halo (<=34 rows of f/g), scans the
recurrence for all loaded rows, and applies its [34 x 18] slice of W
(halo selection + reflection encoded host-side in the weights). No
cross-core communication of any kind.

Performance notes (from neuron-profile traces):
- A dynamic DMA's descriptors are processed by ONE SDMA engine
  (~27 GB/s = one SBUF port); the sync, scalar and gpsimd DGE paths
  are independent, so transfers are split across all three.
- The input is loaded in two waves ordered [f0|g0|a0|W | f1|g1] so the
  first scan half starts while the second half is still in flight.
- Raw Bass (no Tile, no Block): Tile's tail drain exceeds the HW's
  per-instruction sync-wait cap, and Block's exit all-engine barrier
  costs ~4 us of pure epilogue.
"""

import numpy as np

B, C, N = 2, 71, 1024
NCORES = 8
QPB = 4  # channel-quarters per batch element
HALO = 8  # channel reach of the smoothing: steps * (ksz-1)//2
ROWS = 34  # rows loaded per core: own(<=18) + up to 2*HALO, padded
OWN = 18  # owned output channels per core (last quarter uses 17)

_OWN_LO = [0, 18, 36, 54]
_OWN_SZ = [18, 18, 18, 17]

HALF = 512  # one PSUM bank of fp32 per matmul
# Packed input, wave A then wave B: [f0 | g0 | a0 | w] + [f1 | g1]
_F0, _G0, _A0, _W0 = 0, HALF, 2 * HALF, 2 * HALF + 1
_WAVE_B = _W0 + OWN  # 1043
_F1, _G1 = _WAVE_B, _WAVE_B + HALF
PACK = _WAVE_B + 2 * HALF  # 2067

FP32R = True  # single-pass PE matmul (4x faster); flip off if accuracy drops

_PROGRAM = None


def _build_program():
    import concourse.bass as bass
    import concourse.mybir as mybir

    f32 = mybir.dt.float32
    mm_dt = mybir.dt.float32r if FP32R else f32
    mult, add = mybir.AluOpType.mult, mybir.AluOpType.add
    nc = bass.Bass(enable_partition_id=False)
    in_loc = nc.declare_dram_parameter("in_loc", [ROWS, PACK], f32, isOutput=False)
    out_loc = nc.declare_dram_parameter("out_loc", [OWN, N], f32, isOutput=True)

    with (
        nc.sbuf_tensor([ROWS, PACK], f32) as it,
        nc.sbuf_tensor([ROWS, N], mm_dt) as yt,
        nc.sbuf_tensor([ROWS, OWN], mm_dt) as wr,
        nc.sbuf_tensor([OWN, N], f32) as ot,
        nc.psum_tensor([OWN, HALF], f32) as ps0,
        nc.psum_tensor([OWN, HALF], f32) as ps1,
        nc.semaphore("a_hw") as a_hw,  # wave A, HWDGE (sync+scalar)
        nc.semaphore("a_sw") as a_sw,  # wave A, SWDGE (gpsimd)
        nc.semaphore("b_hw") as b_hw,
        nc.semaphore("b_sw") as b_sw,
        nc.semaphore("o_hw") as o_hw,  # output stores
        nc.semaphore("o_sw") as o_sw,
        nc.semaphore("v_sem") as v_sem,
        nc.semaphore("p_sem") as p_sem,
    ):
        a0t = it[:, _A0 : _A0 + 1]
        wt = it[:, _W0 : _W0 + OWN]
        fh = [it[:, _F0 : _F0 + HALF], it[:, _F1 : _F1 + HALF]]
        gh = [it[:, _G0 : _G0 + HALF], it[:, _G1 : _G1 + HALF]]
        ps = [ps0, ps1]

        # Input load: two waves, each split across the three DGE paths.
        SPLITS = [("sync", 0, 12), ("scalar", 12, 24), ("gpsimd", 24, 34)]
        for (c0, c1, hw_sem, sw_sem) in (
            (0, _WAVE_B, a_hw, a_sw),
            (_WAVE_B, PACK, b_hw, b_sw),
        ):
            for eng, r0, r1 in SPLITS:
                getattr(nc, eng).dma_start(
                    out=it[r0:r1, c0:c1], in_=in_loc[r0:r1, c0:c1]
                ).then_inc(sw_sem if eng == "gpsimd" else hw_sem, 16)

        # Recurrence scan in halves, chained via initial=prev_out[:, -1:].
        # The scan writes (and W is staged) in the matmul dtype: the FP32r
        # PE mode needs its inputs produced as FP32r.
        nc.vector.wait_ge(a_hw, 32)
        nc.vector.wait_ge(a_sw, 16)
        nc.vector.tensor_copy(wr[:, :], wt).then_inc(v_sem, 1)
        nc.vector.tensor_tensor_scan(
            yt[:, :HALF], fh[0], gh[0], a0t, op0=mult, op1=add
        ).then_inc(v_sem, 1)
        nc.vector.wait_ge(b_hw, 32)
        nc.vector.wait_ge(b_sw, 16)
        nc.vector.wait_ge(v_sem, 2)  # carry element readable (race detector)
        nc.vector.tensor_tensor_scan(
            yt[:, HALF:],
            fh[1],
            gh[1],
            yt[:, HALF - 1 : HALF],
            op0=mult,
            op1=add,
        ).then_inc(v_sem, 1)

        # Smoothing matmul per half; v_sem >= h+2 implies wave A landed
        # (the DVE gated its ops on it), so wr is ready too.
        for h in range(2):
            nc.tensor.wait_ge(v_sem, h + 2)
            nc.tensor.matmul(
                ps[h][:, :],
                wr[:, :],
                yt[:, h * HALF : (h + 1) * HALF],
                start=True,
                stop=True,
            ).then_inc(p_sem, 1)
            nc.vector.wait_ge(p_sem, h + 1)
            nc.vector.tensor_copy(
                ot[:, h * HALF : (h + 1) * HALF], ps[h][:, :]
            ).then_inc(v_sem, 1)

        # Store, split across the three DGE paths (gated on both copies).
        OUT_SPLITS = [("sync", 0, 6), ("scalar", 6, 12), ("gpsimd", 12, 18)]
        for eng, r0, r1 in OUT_SPLITS:
            e = getattr(nc, eng)
            e.wait_ge(v_sem, 5)
            e.dma_start(out=out_loc[r0:r1, :], in_=ot[r0:r1, :]).then_inc(
                o_sw if eng == "gpsimd" else o_hw, 16
            )
        nc.sync.wait_ge(o_hw, 32)
        nc.gpsimd.wait_ge(o_sw, 16)

    return nc


def _conv_matrix(kernel: np.ndarray, steps: int) -> np.ndarray:
    """[C, C] matrix equivalent to `steps` rounds of symmetric-pad conv."""
    eff = np.array([1.0], np.float64)
    for _ in range(steps):
        eff = np.convolve(eff, kernel.astype(np.float64))
    h = (len(eff) - 1) // 2
    assert h <= HALO, f"kernel reach {h} exceeds layout halo {HALO}"
    W = np.zeros((C, C), np.float64)
    for c in range(C):
        for d in range(-h, h + 1):
            idx = c + d
            if idx < 0:
                idx = -1 - idx
            if idx >= C:
                idx = 2 * C - 1 - idx
            W[idx, c] += eff[d + h]
    return W.astype(np.float32)


def _pack_core(core: int, a_0, f, g, W):
    """Build one core's packed [ROWS, PACK] input; returns (in_loc, b, lo, sz)."""
    b, q = divmod(core, QPB)
    lo, sz = _OWN_LO[q], _OWN_SZ[q]
    r0 = max(0, lo - HALO)
    r1 = min(C, lo + sz + HALO)
    nr = r1 - r0

    in_loc = np.zeros((ROWS, PACK), np.float32)
    in_loc[:, _F0 : _F0 + HALF] = 0.5  # benign f for padded rows
    in_loc[:, _F1 : _F1 + HALF] = 0.5
    in_loc[:nr, _F0 : _F0 + HALF] = f[b, r0:r1, :HALF]
    in_loc[:nr, _F1 : _F1 + HALF] = f[b, r0:r1, HALF:]
    in_loc[:nr, _G0 : _G0 + HALF] = g[b, r0:r1, :HALF]
    in_loc[:nr, _G1 : _G1 + HALF] = g[b, r0:r1, HALF:]
    in_loc[:nr, _A0] = a_0[b, r0:r1]
    in_loc[:nr, _W0 : _W0 + sz] = W[r0:r1, lo : lo + sz]
    return in_loc, b, lo, sz


LAST_RESULT = None  # BassKernelResults of the most recent run (for test.py)
TRACE = False  # set True (e.g. by test.py) to capture an NTFF profile


def kernel(a_0, f, g, kernel, steps):
    global _PROGRAM, LAST_RESULT
    from concourse.bass_utils import run_bass_kernel_spmd

    a_0 = np.asarray(a_0, np.float32)
    f = np.asarray(f, np.float32)
    g = np.asarray(g, np.float32)
    W = _conv_matrix(np.asarray(kernel), int(steps))

    in_maps = []
    meta = []
    for core in range(NCORES):
        in_loc, b, lo, sz = _pack_core(core, a_0, f, g, W)
        in_maps.append({"in_loc": in_loc})
        meta.append((b, lo, sz))

    if _PROGRAM is None:
        _PROGRAM = _build_program()

    res = run_bass_kernel_spmd(
        _PROGRAM, in_maps, core_ids=list(range(NCORES)), trace=TRACE
    )
    LAST_RESULT = res

    out = np.empty((B, C, N), np.float32)
    for core, (b, lo, sz) in enumerate(meta):
        out[b, lo : lo + sz] = res.results[core]["out_loc"][:sz]
    return out
